# revision 1
# baseline (speedup 1.0000x reference)
"""Trainium2 Bass kernel for nn_DPFABase (DPFA knowledge-tracing attention).

Full-input contract: kernel(**inputs) takes the unsharded inputs and returns
the full [B, S] float32 output. Internally: data-parallel over batch across
8 NeuronCores (16 examples per core); the [V, H] embedding / beta / response
tables are replicated.

Per-core pipeline:
  1. Prepass: L2-normalize the embedding table (fp32 in DRAM) into a bf16
     "augmented" table in DRAM scratch: row v = [emb_norm(128) | beta | r0 |
     r1 | padflag | junk...] (256 bf16 = 512 B rows).
  2. Per example pair: one dma_gather (2048 idxs x 512 B) pulls history and
     next rows; PE transposes the embedding chunks to [H, S] layout; QK
     matmul (bf16, fp32 accumulate) -> scoresT [s, q] in PSUM; ACT exp with
     per-partition bias (-k*s + centering; the per-q part of the time-decay
     cancels in softmax); causal mask on the diagonal tile; num/denom
     matmuls against [mastery*pad | pad] -> [q, 2] PSUM.
  3. Final: ability = num/den, sigmoid(ability - beta_next), PE transpose,
     one DMA to the [16, 512] output.
"""
import numpy as np

B, S, H, V = 128, 512, 128, 10000
NCORES = 8
EXC = B // NCORES          # examples per core = 16
NPAIR = EXC // 2           # gather pairs per core = 8
VPAD = 10112               # 79 * 128
NTILES = VPAD // 128       # 79
ROW = 256                  # bf16 elements per augmented table row (512 B)
NEG_COLS = (0, 1, 2, 3)

_CACHE = {}


def _build_nc():
    import os
    import concourse.bacc as bacc
    import concourse.mybir as mybir
    from concourse.tile import TileContext

    STAGE = os.environ.get("KSTAGE", "full")
    NP_RUN = int(os.environ.get("KNPAIR", str(NPAIR)))

    f32 = mybir.dt.float32
    bf16 = mybir.dt.bfloat16
    i16 = mybir.dt.int16
    AF = mybir.ActivationFunctionType
    ALU = mybir.AluOpType

    nc = bacc.Bacc()

    emb = nc.declare_dram_parameter("emb", [VPAD, H], f32, isOutput=False)
    aux = nc.declare_dram_parameter("aux", [VPAD, 4], bf16, isOutput=False)
    idx = nc.declare_dram_parameter("idx", [NPAIR, 2, 128, 64], i16, isOutput=False)
    corr = nc.declare_dram_parameter("corr", [NPAIR, 128, 16], bf16, isOutput=False)
    biaspp = nc.declare_dram_parameter("biaspp", [128, 4], f32, isOutput=False)
    causal = nc.declare_dram_parameter("causal", [128, 128], bf16, isOutput=False)
    identb = nc.declare_dram_parameter("identb", [128, 128], bf16, isOutput=False)
    identf = nc.declare_dram_parameter("identf", [128, 128], f32, isOutput=False)
    out = nc.declare_dram_parameter("out", [EXC, S], f32, isOutput=True)

    augtab = nc.dram_tensor("augtab", [VPAD, ROW], bf16)

    with TileContext(nc) as tc:
        with (
            tc.tile_pool(name="persist", bufs=1) as persist,
            tc.tile_pool(name="pre", bufs=3) as pre,
            tc.tile_pool(name="pres", bufs=2) as pres,
            tc.tile_pool(name="main", bufs=2) as main,
            tc.tile_pool(name="psA", bufs=2, space="PSUM") as psA,
            tc.tile_pool(name="psB", bufs=2, space="PSUM") as psB,
            tc.tile_pool(name="psC", bufs=2, space="PSUM") as psC,
            tc.tile_pool(name="psD", bufs=2, space="PSUM") as psD,
        ):
            # ---------- constants ----------
            bias_t = persist.tile([128, 4], f32, name="bias_t")
            nc.sync.dma_start(out=bias_t[:], in_=biaspp[:, :])
            causal_t = persist.tile([128, 128], bf16, name="causal_t")
            nc.sync.dma_start(out=causal_t[:], in_=causal[:, :])
            identb_t = persist.tile([128, 128], bf16, name="identb_t")
            nc.sync.dma_start(out=identb_t[:], in_=identb[:, :])
            identf_t = persist.tile([128, 128], f32, name="identf_t")
            nc.sync.dma_start(out=identf_t[:], in_=identf[:, :])
            F_all = persist.tile([128, 8 * EXC], f32, name="F_all")
            B_all = persist.tile([128, 4 * EXC], f32, name="B_all")

            # ---------- prepass: normalize table into bf16 augtab ----------
            t = 0
            while t < NTILES:
                gn = min(8, NTILES - t)
                ss_g = pres.tile([128, 8], f32, name="ss_g", tag="ss_g")
                e_tiles = []
                out_tiles = []
                for g in range(gn):
                    e_t = pre.tile([128, H], f32, name="e_t", tag=f"e_t{g}")
                    nc.sync.dma_start(out=e_t[:], in_=emb[(t + g) * 128:(t + g + 1) * 128, :])
                    o_t = pre.tile([128, ROW], bf16, name="o_t", tag=f"o_t{g}")
                    nc.gpsimd.memset(o_t[:, H + 4:ROW], 0.0)
                    nc.sync.dma_start(
                        out=o_t[:, H:H + 4], in_=aux[(t + g) * 128:(t + g + 1) * 128, :]
                    )
                    dump = pre.tile([128, H], bf16, name="dump", tag=f"dump{g}")
                    nc.scalar.activation(
                        dump[:], e_t[:], AF.Square, accum_out=ss_g[:, g:g + 1]
                    )
                    e_tiles.append(e_t)
                    out_tiles.append(o_t)
                sq_g = pres.tile([128, 8], f32, name="sq_g", tag="sq_g")
                nc.scalar.sqrt(sq_g[:, 0:gn], ss_g[:, 0:gn])
                rn_g = pres.tile([128, 8], f32, name="rn_g", tag="rn_g")
                nc.vector.reciprocal(rn_g[:, 0:gn], sq_g[:, 0:gn])
                for g in range(gn):
                    nc.vector.tensor_scalar_mul(
                        out_tiles[g][:, 0:H], e_tiles[g][:], rn_g[:, g:g + 1]
                    )
                    nc.sync.dma_start(
                        out=augtab[(t + g) * 128:(t + g + 1) * 128, :],
                        in_=out_tiles[g][:],
                    )
                t += gn

            tc.strict_bb_all_engine_barrier()

            # ---------- main loop over example pairs ----------
            for k in range(NP_RUN if STAGE != "pre" else 0):  # gather0 skips aux
                G2 = main.tile([128, 16, ROW], bf16, name="G2", tag="G2")
                for kk in range(2):
                    idx_t = main.tile([128, 64], i16, name="idx_t", tag=f"idx_t{kk}")
                    nc.sync.dma_start(out=idx_t[:], in_=idx[k, kk, :, :])
                    nc.gpsimd.dma_gather(
                        G2[:, 8 * kk:8 * kk + 8, :], augtab[:, :], idx_t[:],
                        1024, 1024, ROW, elem_step=ROW,
                    )
                corr_t = main.tile([128, 16], bf16, name="corr_t", tag="corr_t")
                nc.sync.dma_start(out=corr_t[:], in_=corr[k, :, :])
                if STAGE == "gather0":
                    continue

                # mastery / pad lhsT build for both examples of the pair
                c01 = main.tile([128, 16], bf16, name="c01", tag="c01")
                nc.vector.tensor_scalar(
                    out=c01[:], in0=corr_t[:], scalar1=2.0, scalar2=None,
                    op0=ALU.is_equal,
                )
                r0v = G2[:, :, 129]
                r1v = G2[:, :, 130]
                padv = G2[:, :, 131]
                dmt = main.tile([128, 16], bf16, name="dmt", tag="dmt")
                nc.vector.tensor_tensor(out=dmt[:], in0=r1v, in1=r0v, op=ALU.subtract)
                tmt = main.tile([128, 16], bf16, name="tmt", tag="tmt")
                nc.vector.tensor_tensor(out=tmt[:], in0=dmt[:], in1=c01[:], op=ALU.mult)
                mmt = main.tile([128, 16], bf16, name="mmt", tag="mmt")
                nc.vector.tensor_tensor(out=mmt[:], in0=tmt[:], in1=r0v, op=ALU.add)
                T_aux = main.tile([128, 16], bf16, name="T_aux", tag="T_aux")
                # even cols: mastery*pad for hist chunks (0..3, 8..11)
                m_hist = mmt[:].rearrange("p (a b) -> p a b", a=2)[:, :, 0:4]
                pad_hist = G2[:].rearrange("p (a b) r -> p a b r", a=2)[:, :, 0:4, 131]
                T4 = T_aux[:].rearrange("p (a j t) -> p a j t", a=2, j=4)
                nc.vector.tensor_tensor(
                    out=T4[:, :, :, 0], in0=m_hist, in1=pad_hist, op=ALU.mult
                )
                nc.vector.tensor_copy(T4[:, :, :, 1], pad_hist)

                for kk in range(2 if STAGE not in ("gather", "gather0") else 0):
                    e = 2 * k + kk
                    # transposes: hist chunks -> hist_T, next chunks -> next_T
                    psa = psA.tile([128, 512], bf16, name="psa", tag="psa")
                    psb = psB.tile([128, 512], bf16, name="psb", tag="psb")
                    for j in range(4):
                        nc.tensor.transpose(
                            psa[:, 128 * j:128 * (j + 1)],
                            G2[:, 8 * kk + j, 0:H],
                            identb_t[:],
                        )
                        nc.tensor.transpose(
                            psb[:, 128 * j:128 * (j + 1)],
                            G2[:, 8 * kk + 4 + j, 0:H],
                            identb_t[:],
                        )
                    hist_T = main.tile([128, 512], bf16, name="hist_T", tag="hist_T")
                    nc.vector.tensor_copy(hist_T[:], psa[:])
                    next_T = main.tile([128, 512], bf16, name="next_T", tag="next_T")
                    nc.vector.tensor_copy(next_T[:], psb[:])
                    if STAGE == "tr":
                        continue

                    # QK + exp + causal
                    e_tiles2 = []
                    for j in range(4):
                        n_j = 512 - 128 * j
                        sc = psC.tile([128, 512], f32, name="sc", tag=f"sc{j % 2}", bufs=1)
                        nc.tensor.matmul(
                            sc[:, 0:n_j],
                            hist_T[:, 128 * j:128 * (j + 1)],
                            next_T[:, 128 * j:512],
                            start=True, stop=True,
                        )
                        e_j = main.tile([128, 512], bf16, name="e_j", tag=f"e_j{j}")
                        nc.scalar.activation(
                            e_j[:, 0:n_j], sc[:, 0:n_j], AF.Exp,
                            bias=bias_t[:, j:j + 1], scale=1.0,
                        )
                        nc.vector.tensor_tensor(
                            out=e_j[:, 0:128], in0=e_j[:, 0:128], in1=causal_t[:],
                            op=ALU.mult,
                        )
                        e_tiles2.append(e_j)
                    if STAGE == "qk":
                        continue

                    # num/den matmuls: out[q-block c] accumulates over j<=c
                    nd = psD.tile([128, 8], f32, name="nd", tag="nd", bufs=1)
                    for c in range(4):
                        for j in range(c + 1):
                            nc.tensor.matmul(
                                nd[:, 2 * c:2 * c + 2],
                                e_tiles2[j][:, 128 * (c - j):128 * (c - j + 1)],
                                T_aux[:, 8 * kk + 2 * j:8 * kk + 2 * j + 2],
                                start=(j == 0), stop=(j == c),
                            )
                    nc.vector.tensor_copy(F_all[:, 8 * e:8 * e + 8], nd[:])
                    nc.vector.tensor_copy(
                        B_all[:, 4 * e:4 * e + 4], G2[:, 8 * kk + 4:8 * kk + 8, 128]
                    )

            # ---------- finals ----------
            if STAGE == "full":
                F3 = F_all[:].rearrange("p (x t) -> p x t", t=2)
                rd = persist.tile([128, 64], f32, name="rd")
                nc.vector.reciprocal(rd[:], F3[:, :, 1])
                at = persist.tile([128, 64], f32, name="at")
                nc.vector.tensor_tensor(out=at[:], in0=F3[:, :, 0], in1=rd[:], op=ALU.mult)
                zt = persist.tile([128, 64], f32, name="zt")
                nc.vector.tensor_tensor(out=zt[:], in0=at[:], in1=B_all[:], op=ALU.subtract)
                ot = persist.tile([128, 64], f32, name="ot")
                nc.scalar.activation(ot[:], zt[:], AF.Sigmoid)
                pso = psA.tile([128, 128], f32, name="pso", tag="pso", bufs=1)
                nc.tensor.transpose(pso[0:64, :], ot[:], identf_t[:])
                otr = persist.tile([64, 128], f32, name="otr")
                nc.vector.tensor_copy(otr[:], pso[0:64, :])
                nc.sync.dma_start(
                    out=out[:, :].rearrange("e (x q) -> (e x) q", x=4), in_=otr[:]
                )

    nc.finalize()
    return nc


def _marshal(inputs):
    import ml_dtypes

    bf16 = ml_dtypes.bfloat16
    hist = np.asarray(inputs["history_items"]).astype(np.int64)
    nxt = np.asarray(inputs["next_items"]).astype(np.int64)
    corrects = np.asarray(inputs["history_corrects"]).astype(np.int64)
    E = np.asarray(inputs["item_embedding"], dtype=np.float32)
    beta = np.asarray(inputs["item_beta_weights"], dtype=np.float32)
    resp = np.asarray(inputs["item_response_vals"], dtype=np.float32)
    k = float(np.asarray(inputs["td_kernel"]).reshape(-1)[0])

    emb_pad = np.ones((VPAD, H), dtype=np.float32)
    emb_pad[:V] = E

    aux = np.zeros((VPAD, 4), dtype=np.float32)
    aux[:V, 0] = beta
    aux[:V, 1] = resp[:, 0]
    aux[:V, 2] = resp[:, 1]
    aux[:V, 3] = 1.0
    aux[0, 3] = 0.0  # item id 0 is padding
    aux16 = aux.astype(bf16)

    p = np.arange(128, dtype=np.float32)
    biaspp = np.stack(
        [-k * (128.0 * j + p) + k * (S / 2 - 0.5) for j in range(4)], axis=1
    ).astype(np.float32)

    causal = (p[:, None] <= p[None, :]).astype(bf16)  # keep s<=q within tile
    identb = np.eye(128, dtype=np.float32).astype(bf16)
    identf = np.eye(128, dtype=np.float32)

    in_maps = []
    for c in range(NCORES):
        idx_c = np.zeros((NPAIR, 2, 128, 64), dtype=np.int16)
        corr_c = np.zeros((NPAIR, 128, 16), dtype=np.float32)
        for kpair in range(NPAIR):
            e0 = c * EXC + 2 * kpair
            for kk in range(2):
                ids = np.concatenate([hist[e0 + kk], nxt[e0 + kk]]).astype(np.int16)
                w = ids.reshape(64, 16).T  # [16, 64]
                for g in range(8):
                    idx_c[kpair, kk, 16 * g:16 * (g + 1), :] = w
                cseq = corrects[e0 + kk].reshape(4, 128).T  # [128(p), 4(j)]
                corr_c[kpair, :, 8 * kk:8 * kk + 4] = cseq
        in_maps.append(
            dict(
                emb=emb_pad,
                aux=aux16,
                idx=idx_c,
                corr=corr_c.astype(bf16),
                biaspp=biaspp,
                causal=causal,
                identb=identb,
                identf=identf,
            )
        )
    return in_maps


def kernel(**inputs) -> np.ndarray:
    from concourse.bass_utils import run_bass_kernel_spmd

    if "nc" not in _CACHE:
        _CACHE["nc"] = _build_nc()
    nc = _CACHE["nc"]
    in_maps = _marshal(inputs)
    res = run_bass_kernel_spmd(nc, in_maps, list(range(NCORES))).results
    out = np.concatenate([res[c]["out"] for c in range(NCORES)], axis=0)
    return np.ascontiguousarray(out).astype(np.float32)

